# revision 26
# baseline (speedup 1.0000x reference)
"""Trainium2 Bass kernel for nn_BertWordPair (sparse_attention).

Computes: y = x @ W1 + b1 -> split into (q_tok, q_utt, k_tok, k_utt) per
channel c in [0,3); RoPE with block-sign structure from seg_ids; output
logits [B, S, S, 3] = sum over the two groups of the selected-variant
bilinear forms.

Strategy (8 NeuronCores), v2:
  - 2x2 split per batch: each core owns 1024 query rows x 1024 key cols
    of one batch.  This minimizes the replicated projection work: per
    core stage A projects 1024 rows (Q side) + 1024 cols (K side),
    61.4us of PE, vs 77us for the 1D row split.  Stage B (logits) is
    41us.  PE floor ~102us.
  - All matmul inputs and the whole RoPE elementwise pipeline are bf16:
    halves DMA traffic and puts the DVE in its 2x perf mode (all
    operands 2-byte + SBUF).  PSUM accumulation stays fp32; output fp32.
  - Stage-A PSUM is evacuated by the Activation engine (Identity) with
    the per-feature bias fused via the per-partition bias operand,
    writing bf16 to SBUF.  RoPE then is pure tensor_tensor work:
    products (x cos / x sin) on DVE at 2x rate, combines on Pool
    (gpsimd) to balance engine load.
  - All data-dependent signs are folded into host-built tables:
      sigma_q(seg(row), colseg(T)) is folded into the Q sin table
      (varies along rows, one table per col block T);
      sigma_k(rowhalf_seg, seg(col)) into the K sin table (one per row
      half h).  The device kernel has no sign logic at all.
  - Emission is channel-staggered (K-projection of the next channel is
    emitted between stage-B chunks) so the elementwise pipeline always
    runs at least one channel ahead of the PE.
"""
import sys
sys.path.insert(0, '/opt/trn_rl_repo')

import numpy as np

B, S, H, C = 2, 2048, 768, 3
DG = 256              # rope dim per group (tok / utt)
D2 = 512              # feature dim per channel (tok 256 + utt 256)
N_CORES = 8
RPC = 1024            # rows per core
CPC = 1024            # cols per core
BLK = 512             # col block (one stage-B sweep)
NT = CPC // BLK       # 2 col blocks per core
NH = 2                # row halves (512 rows each, one seg per half)
KH = H // 128         # 6 contraction tiles for dense1
FT = (C * D2) // 128  # 12 feature tiles per side (q or k)
NPAIR = FT // 2       # 6 (c, g) pairs per side


def _variant(s, t):
    # 0=PP, 1=NP (q_neg*k_pos), 2=PN (q_pos*k_neg)
    if s >= 1 and t > s:
        return 1
    if t >= 1 and s > t:
        return 2
    return 0


def _freqs(base):
    return np.power(float(base), -2.0 * np.arange(DG // 2, dtype=np.float64) / DG)


def _perm_cols(side_off):
    """New feature order: c*512 + g*256 + p*128 + k  <-  orig
    c*1024 + side_off + g*256 + 2k + p."""
    cols = np.empty(C * D2, np.int64)
    f = 0
    for c in range(C):
        for g in range(2):
            for p in range(2):
                base = c * 1024 + side_off + g * 256 + p
                cols[f:f + 128] = base + 2 * np.arange(128)
                f += 128
    return cols


def _pack_pmajor(a, nt):
    """[nt*128, F] -> [128, nt, F] (partition-major chunks)."""
    F = a.shape[1]
    return np.ascontiguousarray(a.reshape(nt, 128, F).transpose(1, 0, 2))


def _bf16(a):
    import ml_dtypes
    return np.asarray(a, np.float32).astype(ml_dtypes.bfloat16)


def _host_prep(x, W1, b1, token_index, utterance_index, seg_ids):
    """Build per-core input maps + check fast-path validity."""
    x = np.asarray(x, np.float32)
    W1 = np.asarray(W1, np.float32)
    b1 = np.asarray(b1, np.float32)
    token_index = np.asarray(token_index)
    utterance_index = np.asarray(utterance_index)
    seg_ids = np.asarray(seg_ids)

    qcols = _perm_cols(0)     # q_tok at +0, q_utt at +256
    kcols = _perm_cols(512)   # k_tok at +512, k_utt at +768
    WQp = _bf16(_pack_pmajor(np.ascontiguousarray(W1[:, qcols]), KH))
    WKp = _bf16(_pack_pmajor(np.ascontiguousarray(W1[:, kcols]), KH))
    bQ = b1[qcols].astype(np.float32)
    bK = b1[kcols].astype(np.float32)
    biasc = np.ascontiguousarray(
        np.concatenate([bQ, bK]).reshape(2 * FT, 128).T)  # [128, 24]

    xT = x.transpose(0, 2, 1)
    xTp = [_bf16(_pack_pmajor(np.ascontiguousarray(xT[b]), KH)) for b in range(B)]

    fr = [_freqs(10000.0), _freqs(15.0)]  # per group

    in_maps = []
    metas = []
    for core in range(N_CORES):
        b, rh, ch = core // 4, (core // 2) % 2, core % 2
        rows = slice(rh * RPC, (rh + 1) * RPC)
        cols = slice(ch * CPC, (ch + 1) * CPC)
        seg_r = seg_ids[b, rows].astype(np.int64)
        seg_c = seg_ids[b, cols].astype(np.int64)

        # col blocks must each have a uniform segment (Q sin table is per
        # col block); row halves must each have a uniform segment (K sin
        # table is per row half).
        tseg = np.empty(NT, np.int64)
        for T in range(NT):
            blk = seg_c[T * BLK:(T + 1) * BLK]
            if not np.all(blk == blk[0]):
                raise NotImplementedError("fast path: col block must share one seg")
            tseg[T] = blk[0]
        sseg = np.empty(NH, np.int64)
        for h in range(NH):
            half = seg_r[h * 512:(h + 1) * 512]
            if not np.all(half == half[0]):
                raise NotImplementedError("fast path: row half must share one seg")
            sseg[h] = half[0]

        pos_r = [token_index[b, rows].astype(np.float64),
                 utterance_index[b, rows].astype(np.float64)]
        pos_c = [token_index[b, cols].astype(np.float64),
                 utterance_index[b, cols].astype(np.float64)]

        # QCOS [128, 2g, RPC]; QSIN [128, NT, 2g, RPC] (sigma_q folded)
        QCOS = np.empty((128, 2, RPC), np.float64)
        QSIN = np.empty((128, NT, 2, RPC), np.float64)
        sq = np.empty((NT, RPC), np.float64)
        for T in range(NT):
            sq[T] = [-1.0 if _variant(int(s), int(tseg[T])) == 1 else 1.0
                     for s in seg_r]
        for g in range(2):
            ang = fr[g][:, None] * pos_r[g][None, :]
            QCOS[:, g, :] = np.cos(ang)
            for T in range(NT):
                QSIN[:, T, g, :] = sq[T][None, :] * np.sin(ang)

        # KCOS [128, 2g, CPC]; KSIN [128, NH, 2g, CPC] (sigma_k folded)
        KCOS = np.empty((128, 2, CPC), np.float64)
        KSIN = np.empty((128, NH, 2, CPC), np.float64)
        sk = np.empty((NH, CPC), np.float64)
        for h in range(NH):
            sk[h] = [-1.0 if _variant(int(sseg[h]), int(t)) == 2 else 1.0
                     for t in seg_c]
        for g in range(2):
            ang = fr[g][:, None] * pos_c[g][None, :]
            KCOS[:, g, :] = np.cos(ang)
            for h in range(NH):
                KSIN[:, h, g, :] = sk[h][None, :] * np.sin(ang)

        in_maps.append({
            "XQ": np.ascontiguousarray(xTp[b][:, :, rows]),
            "XK": np.ascontiguousarray(xTp[b][:, :, cols]),
            "WQ": WQp, "WK": WKp, "BIASC": biasc,
            "QCOS": _bf16(QCOS), "QSIN": _bf16(QSIN),
            "KCOS": _bf16(KCOS), "KSIN": _bf16(KSIN),
        })
        metas.append({"b": b, "rh": rh, "ch": ch})
    return in_maps, metas


def _build_program(reps=0):
    """Build the SPMD-uniform Bass program."""
    import concourse.bacc as bacc
    import concourse.mybir as mybir
    import concourse.tile as tile
    from contextlib import ExitStack

    f32 = mybir.dt.float32
    bf16 = mybir.dt.bfloat16
    AF = mybir.ActivationFunctionType
    OP = mybir.AluOpType

    nc = bacc.Bacc("TRN2", target_bir_lowering=False, debug=False,
                   num_devices=N_CORES)
    XQd = nc.dram_tensor("XQ", [128, KH, RPC], bf16, kind="ExternalInput")
    XKd = nc.dram_tensor("XK", [128, KH, CPC], bf16, kind="ExternalInput")
    WQd = nc.dram_tensor("WQ", [128, KH, C * D2], bf16, kind="ExternalInput")
    WKd = nc.dram_tensor("WK", [128, KH, C * D2], bf16, kind="ExternalInput")
    BIASC = nc.dram_tensor("BIASC", [128, 2 * FT], f32, kind="ExternalInput")
    QCOSd = nc.dram_tensor("QCOS", [128, 2, RPC], bf16, kind="ExternalInput")
    QSINd = nc.dram_tensor("QSIN", [128, NT, 2, RPC], bf16, kind="ExternalInput")
    KCOSd = nc.dram_tensor("KCOS", [128, 2, CPC], bf16, kind="ExternalInput")
    KSINd = nc.dram_tensor("KSIN", [128, NH, 2, CPC], bf16, kind="ExternalInput")
    OUT = nc.dram_tensor("OUT", [C, RPC, CPC], bf16, kind="ExternalOutput")

    with tile.TileContext(nc) as tc, ExitStack() as ctx:
        wp = ctx.enter_context(tc.tile_pool(name="wp", bufs=2))
        xp = ctx.enter_context(tc.tile_pool(name="xp", bufs=2))
        tabp = ctx.enter_context(tc.tile_pool(name="tabp", bufs=1))
        biasp = ctx.enter_context(tc.tile_pool(name="biasp", bufs=1))
        uvp = ctx.enter_context(tc.tile_pool(name="uvp", bufs=1))
        qtp = ctx.enter_context(tc.tile_pool(name="qtp", bufs=6))
        qep = ctx.enter_context(tc.tile_pool(name="qep", bufs=15))
        ktp = ctx.enter_context(tc.tile_pool(name="ktp", bufs=8))
        kep = ctx.enter_context(tc.tile_pool(name="kep", bufs=30))
        outp = ctx.enter_context(tc.tile_pool(name="outp", bufs=6))
        pap = ctx.enter_context(tc.tile_pool(name="pap", bufs=4, space="PSUM"))
        pbp = ctx.enter_context(tc.tile_pool(name="pbp", bufs=4, space="PSUM"))

        mm = nc.tensor.matmul
        TT = nc.vector.tensor_tensor
        PT = nc.gpsimd.tensor_tensor

        def emit_body():
            bias_all = biasp.tile([128, 2 * FT], f32, name="bias_all",
                                  tag="bias")
            nc.sync.dma_start(bias_all[:], BIASC[:])
            wq = wp.tile([128, KH, C * D2], bf16, name="wq", tag="w")
            nc.sync.dma_start(wq[:], WQd[:])
            xq = xp.tile([128, KH, RPC], bf16, name="xq", tag="x")
            nc.sync.dma_start(xq[:], XQd[:])
            qcos = tabp.tile([128, 2, RPC], bf16, name="qcos", tag="qcos")
            nc.sync.dma_start(qcos[:], QCOSd[:])
            qsin = tabp.tile([128, NT, 2, RPC], bf16, name="qsin", tag="qsin")
            nc.sync.dma_start(qsin[:], QSINd[:])

            # ---------- phase Q: project + evacuate (bias fused) ----------
            uv = []
            for pr in range(NPAIR):
                ft_e, ft_o = 2 * pr, 2 * pr + 1
                u = uvp.tile([128, RPC], bf16, name="u", tag=f"u{pr}")
                v = uvp.tile([128, RPC], bf16, name="v", tag=f"v{pr}")
                for rc in range(2):
                    rsl = slice(rc * 512, (rc + 1) * 512)
                    ps_e = pap.tile([128, 512], f32, name="psa")
                    for kh in range(KH):
                        mm(ps_e[:], wq[:, kh, ft_e * 128:(ft_e + 1) * 128],
                           xq[:, kh, rsl], start=(kh == 0), stop=(kh == KH - 1))
                    ps_o = pap.tile([128, 512], f32, name="psa")
                    for kh in range(KH):
                        mm(ps_o[:], wq[:, kh, ft_o * 128:(ft_o + 1) * 128],
                           xq[:, kh, rsl], start=(kh == 0), stop=(kh == KH - 1))
                    nc.scalar.activation(u[:, rsl], ps_e[:], AF.Identity,
                                         bias=bias_all[:, ft_e:ft_e + 1])
                    nc.scalar.activation(v[:, rsl], ps_o[:], AF.Identity,
                                         bias=bias_all[:, ft_o:ft_o + 1])
                uv.append((u, v))

            # prefetch K-side operands
            wk = wp.tile([128, KH, C * D2], bf16, name="wk", tag="w")
            nc.sync.dma_start(wk[:], WKd[:])
            xk = xp.tile([128, KH, CPC], bf16, name="xk", tag="x")
            nc.sync.dma_start(xk[:], XKd[:])
            kcos = tabp.tile([128, 2, CPC], bf16, name="kcos", tag="kcos")
            nc.sync.dma_start(kcos[:], KCOSd[:])
            ksin = tabp.tile([128, NH, 2, CPC], bf16, name="ksin", tag="ksin")
            nc.sync.dma_start(ksin[:], KSINd[:])

            def emit_qe(t, c):
                """Q_eff tiles for channel c against col block t (DVE)."""
                res = []
                for g in range(2):
                    u, v = uv[c * 2 + g]
                    ae = qtp.tile([128, RPC], bf16, name="ae", tag="qt")
                    TT(ae[:], u[:], qcos[:, g, :], OP.mult)
                    sv = qtp.tile([128, RPC], bf16, name="sv", tag="qt")
                    TT(sv[:], v[:], qsin[:, t, g, :], OP.mult)
                    qe_e = qep.tile([128, RPC], bf16, name="qe", tag="qe")
                    TT(qe_e[:], ae[:], sv[:], OP.subtract)
                    ao = qtp.tile([128, RPC], bf16, name="ao", tag="qt")
                    TT(ao[:], v[:], qcos[:, g, :], OP.mult)
                    su = qtp.tile([128, RPC], bf16, name="su", tag="qt")
                    TT(su[:], u[:], qsin[:, t, g, :], OP.mult)
                    qe_o = qep.tile([128, RPC], bf16, name="qe", tag="qe")
                    TT(qe_o[:], ao[:], su[:], OP.add)
                    res += [qe_e, qe_o]
                return res

            def emit_a(t, c):
                """K proj + rope for channel c, col block t -> ke[h][dt]."""
                csl = slice(t * BLK, (t + 1) * BLK)
                ke = [[], []]
                for g in range(2):
                    ft_e, ft_o = c * 4 + g * 2, c * 4 + g * 2 + 1
                    ps_e = pap.tile([128, BLK], f32, name="psa")
                    for kh in range(KH):
                        mm(ps_e[:], wk[:, kh, ft_e * 128:(ft_e + 1) * 128],
                           xk[:, kh, csl], start=(kh == 0), stop=(kh == KH - 1))
                    ps_o = pap.tile([128, BLK], f32, name="psa")
                    for kh in range(KH):
                        mm(ps_o[:], wk[:, kh, ft_o * 128:(ft_o + 1) * 128],
                           xk[:, kh, csl], start=(kh == 0), stop=(kh == KH - 1))
                    uk = ktp.tile([128, BLK], bf16, name="uk", tag="kt")
                    nc.scalar.activation(uk[:], ps_e[:], AF.Identity,
                                         bias=bias_all[:, FT + ft_e:FT + ft_e + 1])
                    vk = ktp.tile([128, BLK], bf16, name="vk", tag="kt")
                    nc.scalar.activation(vk[:], ps_o[:], AF.Identity,
                                         bias=bias_all[:, FT + ft_o:FT + ft_o + 1])
                    ak_e = ktp.tile([128, BLK], bf16, name="ake", tag="kt")
                    TT(ak_e[:], uk[:], kcos[:, g, csl], OP.mult)
                    ak_o = ktp.tile([128, BLK], bf16, name="ako", tag="kt")
                    TT(ak_o[:], vk[:], kcos[:, g, csl], OP.mult)
                    for h in range(NH):
                        sv = ktp.tile([128, BLK], bf16, name="svk", tag="kt")
                        TT(sv[:], vk[:], ksin[:, h, g, csl], OP.mult)
                        su = ktp.tile([128, BLK], bf16, name="suk", tag="kt")
                        TT(su[:], uk[:], ksin[:, h, g, csl], OP.mult)
                        ke_e = kep.tile([128, BLK], bf16, name="ke", tag="ke")
                        PT(ke_e[:], ak_e[:], sv[:], OP.subtract)
                        ke_o = kep.tile([128, BLK], bf16, name="ke", tag="ke")
                        PT(ke_o[:], ak_o[:], su[:], OP.add)
                        ke[h] += [ke_e, ke_o]
                return ke

            def emit_b(t, c, qe_c, ke_c):
                """Stage B for channel c, col block t."""
                csl = slice(t * BLK, (t + 1) * BLK)
                for h in range(NH):
                    for m in range(4):
                        rlo = h * 512 + m * 128
                        pb = pbp.tile([128, BLK], f32, name="psb")
                        for dt in range(4):
                            mm(pb[:], qe_c[dt][:, rlo:rlo + 128],
                               ke_c[h][dt][:],
                               start=(dt == 0), stop=(dt == 3))
                        ob = outp.tile([128, BLK], bf16, name="ob", tag="ob")
                        nc.scalar.activation(ob[:], pb[:], AF.Identity)
                        nc.sync.dma_start(OUT[c, rlo:rlo + 128, csl], ob[:])

            # ---------- staggered block/channel pipeline ----------
            # (qe, a) one channel ahead of b
            work = [(t, c) for t in range(NT) for c in range(C)]
            staged = {}
            qe_s = {}
            qe_s[work[0]] = emit_qe(*work[0])
            staged[work[0]] = emit_a(*work[0])
            for i, (t, c) in enumerate(work):
                if i + 1 < len(work):
                    nxt = work[i + 1]
                    qe_s[nxt] = emit_qe(*nxt)
                    staged[nxt] = emit_a(*nxt)
                emit_b(t, c, qe_s.pop((t, c)), staged.pop((t, c)))

        # Unroll 4 bodies per loop iteration: quarters the per-rep cost of
        # the For_i all-engine barrier and lets body i+1's input DMAs
        # overlap body i's tail through the rotating tile pools.
        if reps and reps > 1:
            U = 4
            n, rem = divmod(reps, U)
            if n > 0:
                with tc.For_i(0, n, 1, staggered_reset=True):
                    for _ in range(U):
                        emit_body()
                if rem > 1:
                    with tc.For_i(0, rem, 1):
                        emit_body()
                elif rem == 1:
                    emit_body()
            else:
                for _ in range(rem):
                    emit_body()
        else:
            emit_body()

    nc.compile()
    return nc


_PROG_CACHE = {}


def kernel(**inputs):
    from concourse.bass_utils import run_bass_kernel_spmd

    in_maps, metas = _host_prep(**inputs)
    if "prog" not in _PROG_CACHE:
        _PROG_CACHE["prog"] = _build_program()
    nc = _PROG_CACHE["prog"]

    res = run_bass_kernel_spmd(nc, in_maps, list(range(N_CORES)))
    out = np.empty((B, S, S, C), np.float32)
    for core in range(N_CORES):
        m = metas[core]
        o = np.asarray(res.results[core]["OUT"], np.float32)  # [C, RPC, CPC]
        out[m["b"], m["rh"] * RPC:(m["rh"] + 1) * RPC,
            m["ch"] * CPC:(m["ch"] + 1) * CPC] = o.transpose(1, 2, 0)
    return out
